# revision 9
# baseline (speedup 1.0000x reference)
"""Rotated-quad IoU loss on Trainium2, 8-core data parallel.

Replicates the reference polygon-intersection IoU algorithm exactly:
16 edge-edge intersection candidates + 8 vertex-in-quad candidates,
centroid, angular sort (order-equivalent pseudoangle instead of atan2),
shoelace area, IoU, mean(1 - IoU).

Layout: batch is sharded 8 ways (16 images/core).  Each core processes its
16*1024 = 16384 quad pairs as [128 partitions x 128 free] f32 planes, one
plane per scalar quantity (struct-of-arrays).  Pair index = p*128 + f;
image index = p >> 3 (constant per partition, so the per-image [w,h] scale
is a per-partition broadcast).
"""

import numpy as np

P, F = 128, 128
NCORES = 8
B, N = 128, 1024
B_LOC = B // NCORES            # 16 images per core
EPS_DEN = 1e-9
EPS_IN = 1e-6
BIG = 1e9
BIG_THRESH = 1e8
REGS = 220                     # shared SBUF plane slots (512B/partition each)


# ---------------------------------------------------------------------------
# Batcher odd-even merge sort network, pruned to n inputs
# ---------------------------------------------------------------------------
def batcher_pairs(n):
    p2 = 1
    while p2 < n:
        p2 *= 2
    pairs = []

    def merge(lo, hi, r):          # inclusive bounds, power-of-two span
        step = r * 2
        if step < hi - lo:
            merge(lo, hi, step)
            merge(lo + r, hi, step)
            for i in range(lo + r, hi - r, step):
                pairs.append((i, i + r))
        else:
            pairs.append((lo, lo + r))

    def sort(lo, hi):
        if hi - lo >= 1:
            mid = lo + (hi - lo) // 2
            sort(lo, mid)
            sort(mid + 1, hi)
            merge(lo, hi, 1)

    sort(0, p2 - 1)
    return [(a, b) for (a, b) in pairs if b < n]


SORT_NET = batcher_pairs(24)


# ---------------------------------------------------------------------------
# Emitters: the same algorithm runs on numpy planes (validation) or emits
# Bass instructions (device).
# ---------------------------------------------------------------------------
class NpEmit:
    """Numpy fp32 plane emitter mirroring device ALU semantics."""

    def __init__(self):
        self.n_ops = 0

    def _f(self, x):
        return np.asarray(x, np.float32)

    def add(self, a, b):
        self.n_ops += 1
        return self._f(a + b)

    def sub(self, a, b):
        self.n_ops += 1
        return self._f(a - b)

    def mul(self, a, b):
        self.n_ops += 1
        return self._f(a * b)

    def emin(self, a, b):
        self.n_ops += 1
        return np.fmin(a, b)

    def emax(self, a, b):
        self.n_ops += 1
        return np.fmax(a, b)

    def gt(self, a, b):
        self.n_ops += 1
        return self._f(a > b)

    def gti(self, a, b):
        """is_gt producing an int mask (device: int32 tile for predication)."""
        self.n_ops += 1
        return self._f(a > b)

    def ltci(self, a, c):
        self.n_ops += 1
        return self._f(a < np.float32(c))

    def blend(self, m, t, fc):
        """m*t + (1-m)*fc, exact for m in {0,1}: select without predication."""
        self.n_ops += 3
        a = self._f(m * t)
        b = self._f(self._f(m * np.float32(-fc)) + np.float32(fc))
        return self._f(a + b)

    # tensor_scalar ops (const is python float; fp32 semantics)
    def addc(self, a, c):
        self.n_ops += 1
        return self._f(a + np.float32(c))

    def mulc(self, a, c):
        self.n_ops += 1
        return self._f(a * np.float32(c))

    def maxc(self, a, c):
        self.n_ops += 1
        return np.fmax(a, np.float32(c))

    def gec(self, a, c):
        self.n_ops += 1
        return self._f(a >= np.float32(c))

    def lec(self, a, c):
        self.n_ops += 1
        return self._f(a <= np.float32(c))

    def gtc(self, a, c):
        self.n_ops += 1
        return self._f(a > np.float32(c))

    def ltc(self, a, c):
        self.n_ops += 1
        return self._f(a < np.float32(c))

    def ts2(self, a, s1, op0, s2, op1):
        """(a op0 s1) op1 s2, ops in {mult, add}."""
        self.n_ops += 1
        r = a * np.float32(s1) if op0 == "mult" else a + np.float32(s1)
        r = self._f(r)
        r = r * np.float32(s2) if op1 == "mult" else r + np.float32(s2)
        return self._f(r)

    # (a op0 c) op1 b  -- scalar_tensor_tensor
    def stt(self, a, c, op0, op1, b):
        self.n_ops += 1
        lhs = {
            "is_ge": lambda x: self._f(x >= np.float32(c)),
            "is_le": lambda x: self._f(x <= np.float32(c)),
            "is_gt": lambda x: self._f(x > np.float32(c)),
            "is_lt": lambda x: self._f(x < np.float32(c)),
            "mult": lambda x: self._f(x * np.float32(c)),
            "add": lambda x: self._f(x + np.float32(c)),
        }[op0](a)
        rhs = {
            "mult": lambda x, y: self._f(x * y),
            "add": lambda x, y: self._f(x + y),
            "subtract": lambda x, y: self._f(x - y),
        }[op1](lhs, b)
        return rhs

    def babs(self, a):
        self.n_ops += 1
        return np.abs(a)

    def recip(self, a):
        self.n_ops += 1
        return self._f(np.float32(1.0) / a)

    def sel(self, m, t, f):
        self.n_ops += 2
        return np.where(m != 0, t, f)

    def selc(self, m, t, fc):
        self.n_ops += 2
        return np.where(m != 0, t, np.float32(fc))

    def reduce_free(self, a):
        self.n_ops += 1
        return np.sum(a, axis=1, dtype=np.float32, keepdims=True)


class BassEmit:
    """Emits Bass/Tile instructions; values are SBUF [128,128] f32 tiles."""

    def __init__(self, nc, tc, pool, mybir):
        self.nc = nc
        self.tc = tc
        self.pool = pool
        self.mybir = mybir
        self.k = 0
        self.n_ops = 0

    def _new(self):
        self.k += 1
        t = self.pool.tile([P, F], self.mybir.dt.float32, tag="reg",
                           name=f"v{self.k}")
        return t

    def _alu(self, name):
        return getattr(self.mybir.AluOpType, name)

    def tt(self, op, a, b):
        o = self._new()
        self.nc.any.tensor_tensor(o[:], a[:], b[:], self._alu(op))
        self.n_ops += 1
        return o

    def add(self, a, b):
        return self.tt("add", a, b)

    def sub(self, a, b):
        return self.tt("subtract", a, b)

    def mul(self, a, b):
        return self.tt("mult", a, b)

    def emin(self, a, b):
        return self.tt("min", a, b)

    def emax(self, a, b):
        return self.tt("max", a, b)

    def gt(self, a, b):
        return self.tt("is_gt", a, b)

    def _new_i(self):
        self.k += 1
        t = self.pool.tile([P, F], self.mybir.dt.int32, tag="reg",
                           name=f"v{self.k}")
        return t

    def gti(self, a, b):
        o = self._new_i()
        self.nc.any.tensor_tensor(o[:], a[:], b[:], self._alu("is_gt"))
        self.n_ops += 1
        return o

    def ltci(self, a, c):
        o = self._new_i()
        self.nc.any.tensor_scalar(o[:], a[:], float(c), None,
                                  self._alu("is_lt"))
        self.n_ops += 1
        return o

    def blend(self, m, t, fc):
        a = self.mul(m, t)
        b = self.ts2(m, -float(fc), "mult", float(fc), "add")
        return self.add(a, b)

    def ts(self, op, a, c):
        o = self._new()
        self.nc.any.tensor_scalar(o[:], a[:], float(c), None, self._alu(op))
        self.n_ops += 1
        return o

    def addc(self, a, c):
        return self.ts("add", a, c)

    def mulc(self, a, c):
        return self.ts("mult", a, c)

    def maxc(self, a, c):
        return self.ts("max", a, c)

    def gec(self, a, c):
        return self.ts("is_ge", a, c)

    def lec(self, a, c):
        return self.ts("is_le", a, c)

    def gtc(self, a, c):
        return self.ts("is_gt", a, c)

    def ltc(self, a, c):
        return self.ts("is_lt", a, c)

    def ts2(self, a, s1, op0, s2, op1):
        o = self._new()
        self.nc.any.tensor_scalar(o[:], a[:], float(s1), float(s2),
                                  self._alu(op0), self._alu(op1))
        self.n_ops += 1
        return o

    def stt(self, a, c, op0, op1, b):
        o = self._new()
        self.nc.vector.scalar_tensor_tensor(
            o[:], a[:], float(c), b[:], self._alu(op0), self._alu(op1))
        self.n_ops += 1
        return o

    def babs(self, a):
        o = self._new()
        self.nc.scalar.activation(
            o[:], a[:], self.mybir.ActivationFunctionType.Abs)
        self.n_ops += 1
        return o

    def recip(self, a):
        o = self._new()
        self.nc.vector.reciprocal(o[:], a[:])
        self.n_ops += 1
        return o

    def sel(self, m, t, f):
        o = self._new()
        self.nc.any.tensor_copy(o[:], f[:])
        self.nc.vector.copy_predicated(o[:], m[:], t[:])
        self.n_ops += 2
        return o

    def selc(self, m, t, fc):
        o = self._new()
        self.nc.any.memset(o[:], float(fc))
        self.nc.vector.copy_predicated(o[:], m[:], t[:])
        self.n_ops += 2
        return o

    def reduce_free(self, a):
        o = self.pool.tile([P, 1], self.mybir.dt.float32, tag="redout",
                           name="partial_sb")
        self.nc.vector.tensor_reduce(
            o[:], a[:], self.mybir.AxisListType.X, self._alu("add"))
        self.n_ops += 1
        return o


# ---------------------------------------------------------------------------
# The IoU-loss computation, emitter-agnostic.
# gx, gy, dx, dy: lists of 4 planes each (gt / det quad vertex coordinates,
# gt already scaled to pixels).  Returns the (1 - iou) plane.
# ---------------------------------------------------------------------------
def emit_iou(E, gx, gy, dx, dy):
    def tree_sum(vals):
        vals = list(vals)
        while len(vals) > 1:
            nxt = []
            for i in range(0, len(vals) - 1, 2):
                nxt.append(E.add(vals[i], vals[i + 1]))
            if len(vals) % 2:
                nxt.append(vals[-1])
            vals = nxt
        return vals[0]

    # quad edges
    e1x = [E.sub(gx[(i + 1) % 4], gx[i]) for i in range(4)]
    e1y = [E.sub(gy[(i + 1) % 4], gy[i]) for i in range(4)]
    e2x = [E.sub(dx[(j + 1) % 4], dx[j]) for j in range(4)]
    e2y = [E.sub(dy[(j + 1) % 4], dy[j]) for j in range(4)]

    pts_x, pts_y, msk = [], [], []

    # 16 edge-edge intersection candidates (i: gt edge, j: det edge)
    for i in range(4):
        for j in range(4):
            den = E.sub(E.mul(e1x[i], e2y[j]), E.mul(e1y[i], e2x[j]))
            rx = E.sub(dx[j], gx[i])
            ry = E.sub(dy[j], gy[i])
            tn = E.sub(E.mul(rx, e2y[j]), E.mul(ry, e2x[j]))
            un = E.sub(E.mul(rx, e1y[i]), E.mul(ry, e1x[i]))
            ad = E.babs(den)
            ok = E.gtc(ad, EPS_DEN)
            safe = E.blend(ok, den, 1.0)
            rs = E.recip(safe)
            t = E.mul(tn, rs)
            u = E.mul(un, rs)
            v = E.stt(t, 0.0, "is_ge", "mult", ok)
            v = E.stt(t, 1.0, "is_le", "mult", v)
            v = E.stt(u, 0.0, "is_ge", "mult", v)
            v = E.stt(u, 1.0, "is_le", "mult", v)
            ix = E.add(gx[i], E.mul(t, e1x[i]))
            iy = E.add(gy[i], E.mul(t, e1y[i]))
            pts_x.append(ix)
            pts_y.append(iy)
            msk.append(v)

    # vertex-in-quad candidates; cr[j] = cross(edge_j, q - poly_j)
    def in_poly(qx, qy, px, py, ex, ey):
        ins = []
        for m in range(4):
            crs = []
            for j in range(4):
                ddy = E.sub(qy[m], py[j])
                ddx = E.sub(qx[m], px[j])
                crs.append(E.sub(E.mul(ex[j], ddy), E.mul(ey[j], ddx)))
            ap = E.gec(crs[0], -EPS_IN)
            for c in crs[1:]:
                ap = E.stt(c, -EPS_IN, "is_ge", "mult", ap)
            an = E.lec(crs[0], EPS_IN)
            for c in crs[1:]:
                an = E.stt(c, EPS_IN, "is_le", "mult", an)
            ins.append(E.emax(ap, an))
        return ins

    msk += in_poly(gx, gy, dx, dy, e2x, e2y)   # gt verts in det (order:
    msk_in2 = in_poly(dx, dy, gx, gy, e1x, e1y)  # computed after to keep
    msk += msk_in2                               # candidate order [val,in1,in2]
    pts_x += gx + dx
    pts_y += gy + dy

    # centroid of valid candidates
    cnt = tree_sum(msk)
    sx = tree_sum([E.mul(msk[k], pts_x[k]) for k in range(24)])
    sy = tree_sum([E.mul(msk[k], pts_y[k]) for k in range(24)])
    rc = E.recip(E.maxc(cnt, 1.0))
    cx = E.mul(sx, rc)
    cy = E.mul(sy, rc)

    # pseudoangle keys: sign(ry) * (1 - rx/(|rx|+|ry|)), same order as atan2
    keys = []
    for k in range(24):
        rx = E.sub(pts_x[k], cx)
        ry = E.sub(pts_y[k], cy)
        s = E.add(E.babs(rx), E.babs(ry))
        rs = E.recip(E.maxc(s, 1e-30))
        q = E.mul(rx, rs)
        om = E.ts2(q, -1.0, "mult", 1.0, "add")        # 1 - q in [0, 2]
        sg = E.stt(ry, 0.0, "is_ge", "mult", om)        # om if ry>=0 else 0
        kr = E.stt(sg, 2.0, "mult", "subtract", om)     # om*(2*[ry>=0]-1)
        keys.append(E.blend(msk[k], kr, BIG))
    pts_x = list(pts_x)
    pts_y = list(pts_y)

    # sort 24 (key, x, y) triples by key
    for (a, b) in SORT_NET:
        s = E.gti(keys[a], keys[b])
        kl = E.emin(keys[a], keys[b])
        kh = E.emax(keys[a], keys[b])
        keys[a], keys[b] = kl, kh
        xa = E.sel(s, pts_x[b], pts_x[a])
        xb = E.sel(s, pts_x[a], pts_x[b])
        pts_x[a], pts_x[b] = xa, xb
        ya = E.sel(s, pts_y[b], pts_y[a])
        yb = E.sel(s, pts_y[a], pts_y[b])
        pts_y[a], pts_y[b] = ya, yb

    # replace invalid (key=BIG, sorted last) with first point
    for k in range(1, 24):
        smk = E.ltci(keys[k], BIG_THRESH)
        pts_x[k] = E.sel(smk, pts_x[k], pts_x[0])
        pts_y[k] = E.sel(smk, pts_y[k], pts_y[0])

    # shoelace area of the sorted 24-gon
    terms = []
    for k in range(24):
        kn = (k + 1) % 24
        terms.append(E.sub(E.mul(pts_x[k], pts_y[kn]),
                           E.mul(pts_x[kn], pts_y[k])))
    inter = E.mulc(E.babs(tree_sum(terms)), 0.5)
    inter = E.stt(cnt, 3.0, "is_ge", "mult", inter)     # cnt>=3 gate

    def quad_area(xs, ys):
        t4 = [E.sub(E.mul(xs[i], ys[(i + 1) % 4]),
                    E.mul(xs[(i + 1) % 4], ys[i])) for i in range(4)]
        return E.mulc(E.babs(tree_sum(t4)), 0.5)

    a1 = quad_area(gx, gy)
    a2 = quad_area(dx, dy)
    union = E.addc(E.sub(E.add(a1, a2), inter), EPS_DEN)
    iou = E.mul(inter, E.recip(union))
    return E.ts2(iou, -1.0, "mult", 1.0, "add")          # 1 - iou


# ---------------------------------------------------------------------------
# numpy reference path (used by test.py to validate the algorithm)
# ---------------------------------------------------------------------------
def numpy_core_loss_planes(gt_t, det_t, wcol, hcol):
    """gt_t/det_t: [8,128,128] SoA planes; wcol/hcol: [128,1]. Returns
    (1-iou) plane [128,128] plus op count."""
    E = NpEmit()
    gx = [np.float32(gt_t[2 * i] * wcol) for i in range(4)]
    gy = [np.float32(gt_t[2 * i + 1] * hcol) for i in range(4)]
    dx = [det_t[2 * j] for j in range(4)]
    dy = [det_t[2 * j + 1] for j in range(4)]
    return emit_iou(E, gx, gy, dx, dy), E.n_ops


# ---------------------------------------------------------------------------
# Host-side sharding
# ---------------------------------------------------------------------------
def shard_inputs(gt_boxes, det_boxes, sizes):
    """Build per-core input maps (SoA planes + per-partition scale cols)."""
    gt = np.ascontiguousarray(gt_boxes, np.float32).reshape(B, N, 8)
    det = np.ascontiguousarray(det_boxes, np.float32).reshape(B, N, 8)
    sizes = np.asarray(sizes)
    w = sizes[:, 1].astype(np.float32)     # size = [h, w]
    h = sizes[:, 0].astype(np.float32)
    maps = []
    for c in range(NCORES):
        sl = slice(c * B_LOC, (c + 1) * B_LOC)
        gt_t = np.ascontiguousarray(
            gt[sl].reshape(B_LOC * N, 8).T.reshape(8, P, F))
        det_t = np.ascontiguousarray(
            det[sl].reshape(B_LOC * N, 8).T.reshape(8, P, F))
        # partition p holds pairs of image (p >> 3) within the shard
        wcol = np.repeat(w[sl], P // B_LOC).reshape(P, 1)
        hcol = np.repeat(h[sl], P // B_LOC).reshape(P, 1)
        maps.append({"gt": gt_t, "det": det_t,
                     "wcol": np.ascontiguousarray(wcol),
                     "hcol": np.ascontiguousarray(hcol)})
    return maps


# ---------------------------------------------------------------------------
# Bass kernel construction (cached)
# ---------------------------------------------------------------------------
_CACHE = {}


def build_bass():
    if "nc" in _CACHE:
        return _CACHE["nc"]
    import concourse.bacc as bacc
    import concourse.mybir as mybir
    from concourse.tile import TileContext

    nc = bacc.Bacc()
    gt_d = nc.declare_dram_parameter("gt", [8, P, F], mybir.dt.float32,
                                     isOutput=False)
    det_d = nc.declare_dram_parameter("det", [8, P, F], mybir.dt.float32,
                                      isOutput=False)
    wcol_d = nc.declare_dram_parameter("wcol", [P, 1], mybir.dt.float32,
                                       isOutput=False)
    hcol_d = nc.declare_dram_parameter("hcol", [P, 1], mybir.dt.float32,
                                       isOutput=False)
    out_d = nc.declare_dram_parameter("partial", [P, 1], mybir.dt.float32,
                                      isOutput=True)

    with TileContext(nc) as tc:
        with tc.tile_pool(name="regs", bufs=REGS) as pool, \
             tc.tile_pool(name="cols", bufs=1) as colpool:
            E = BassEmit(nc, tc, pool, mybir)
            wcol = colpool.tile([P, 1], mybir.dt.float32, tag="wcol",
                                name="wcol_sb")
            hcol = colpool.tile([P, 1], mybir.dt.float32, tag="hcol",
                                name="hcol_sb")
            nc.sync.dma_start(out=wcol[:], in_=wcol_d[:])
            nc.sync.dma_start(out=hcol[:], in_=hcol_d[:])

            g_pl, d_pl = [], []
            for c in range(8):
                tg = pool.tile([P, F], mybir.dt.float32, tag="reg",
                               name=f"gin{c}")
                nc.sync.dma_start(out=tg[:], in_=gt_d[c])
                g_pl.append(tg)
                td = pool.tile([P, F], mybir.dt.float32, tag="reg",
                               name=f"din{c}")
                nc.sync.dma_start(out=td[:], in_=det_d[c])
                d_pl.append(td)

            def scale_by(col, plane):
                o = E._new()
                nc.any.tensor_scalar(o[:], plane[:], col[:], None,
                                     mybir.AluOpType.mult)
                return o

            gx = [scale_by(wcol, g_pl[2 * i]) for i in range(4)]
            gy = [scale_by(hcol, g_pl[2 * i + 1]) for i in range(4)]
            dx = [d_pl[2 * j] for j in range(4)]
            dy = [d_pl[2 * j + 1] for j in range(4)]

            one_m = emit_iou(E, gx, gy, dx, dy)
            partial = E.reduce_free(one_m)
            nc.sync.dma_start(out=out_d[:], in_=partial[:])

    if not nc.is_finalized():
        nc.finalize()
    _CACHE["nc"] = nc
    return nc


def run_on_device(in_maps, trace=False, **kw):
    from concourse.bass_utils import run_bass_kernel_spmd
    nc = build_bass()
    res = run_bass_kernel_spmd(nc, in_maps, list(range(NCORES)), trace=trace,
                               **kw)
    return res


def kernel(gt_boxes, det_boxes, sizes):
    in_maps = shard_inputs(gt_boxes, det_boxes, sizes)
    res = run_on_device(in_maps)
    total = np.float64(0.0)
    for r in res.results:
        total += np.sum(r["partial"].astype(np.float64))
    return np.float32(total / (B * N))
